# revision 13
# baseline (speedup 1.0000x reference)
"""Trainium2 Bass kernel: conv2d(3x3,VALID) + bias -> min over C_out -> tanh(tanh).

Full-input contract: kernel(**inputs) takes the unsharded inputs
  x:           [32, 16, 256, 256] f32
  conv_weight: [64, 16, 3, 3]     f32
  conv_bias:   [64]               f32
and returns [32, 1, 254, 254] f32.

Strategy (data-parallel over batch, 4 images per core on 8 cores):
The conv is cast as matmuls over a "kw-shifted slab" in SBUF:
  slab[kw*16+c, t] = x[c, t+kw]   (kw in 0..6, flattened image cols t)
plus a ones row (partition 112) that carries the bias through the matmul.
For a block of 640 flat positions p = base + 5*m + j (m in 0..127, j in 0..4):
  out[m, (j,o)] = sum_kh  slab[:, base+kh*256+5m].T @ wmov[kh]
with wmov[kh][kw*16+c, j*64+o] = W[o,c,kh,kw-j] (0 when kw-j not in 0..2).
This yields PSUM [128 positions, 5 shifts, 64 channels]; the channel-min is
then a free-dim reduce_min on DVE, followed by tanh(tanh()) on ACT.
Computed on the full 256-wide rows; the host drops the 2 garbage columns/rows.

Numerics: the slab is stored in fp8 E3M4 (x ~ N(0,1) fits the +-15.5 range;
4 mantissa bits measure 1.53e-2 final rel err on the actual jax key(0)
inputs vs the 2e-2 budget).  This halves DMA bytes per partition, which is
the wall: all 16 SDMA engines run at ~13-17 GB/s each with all 8 cores
DMAing concurrently, and even AXI ports (partitions 0..63) carry 8 slab
rows each.  Weights stay fp16 (mixed-dtype matmul; the PE upconverts each
operand independently -- verified exact on HW).

Schedule: the Tile DMA->compute dependency is per-tile, so image 0 ramps up
through small windows (2, 8, 41 blocks) to start the first matmul ~12us in
instead of ~26us (full-window DMA wait).  Keep DMAs single-segment
contiguous: multi-segment APs cost ~2.5-3.2us of HWDGE descriptor
generation each vs ~0.6us contiguous (measured).  PSUM reduces are batched
4 blocks per DVE instruction; outputs staged per window, one DMA each.
"""

import sys
import types

import numpy as np
import ml_dtypes

# ---------------------------------------------------------------------------
# NTFF profile hook registration (the container's antenv stub lacks
# axon_hooks; registering it enables trace=True for profiling runs).
def _install_axon_hooks():
    try:
        import antenv.axon_hooks  # noqa: F401
        return
    except ImportError:
        pass
    try:
        import antenv
        from trn_agent_boot.trn_boot import _ntff_profile_via_ctypes
    except ImportError:
        return
    mod = types.ModuleType("antenv.axon_hooks")
    _hook = [None]
    mod.set_axon_ntff_profile_hook = lambda h: _hook.__setitem__(0, h)
    mod.get_axon_ntff_profile_hook = lambda: _hook[0]
    sys.modules["antenv.axon_hooks"] = mod
    antenv.axon_hooks = mod
    try:
        mod.set_axon_ntff_profile_hook(
            _ntff_profile_via_ctypes("/opt/axon/libaxon_pjrt.so")
        )
    except Exception:
        pass


_install_axon_hooks()

import concourse.bass as bass  # noqa: E402
import concourse.tile as tile  # noqa: E402
from concourse import bacc, mybir  # noqa: E402
from concourse.bass_utils import run_bass_kernel_spmd  # noqa: E402

N_CORES = 8
IMGS_PER_CORE = 4
C_IN, H, W = 16, 256, 256
C_OUT = 64
OH = OW = 254

J = 5                 # position shifts per matmul column group
WK = 7                # kw taps present in the contraction (0..6)
KDIM = WK * C_IN + 1  # 113 partitions: 7 kw-shifts x 16 ch + ones row
NFREE = J * C_OUT     # 320 moving columns
BLK = 128 * J         # 640 flat positions per block
IMG = H * W           # 65536
PAD_COLS = 66560      # 260 rows of 256 (2 extra conv rows + slack)
NBLOCKS = 102         # blocks 0..101 cover flat positions 0..65279

# extra cols a window needs past its blocks: the last block's kh=2 lhsT
# slice spans cols [(nb-1)*640 + 512, nb*640 + 512)
CEXTRA = 512
GRP = 4               # blocks per PSUM tile / DVE reduce

# windows: (start_block, n_blocks) per image; image 0 ramps up small so the
# first matmul isn't gated on a full-window DMA (per-tile dependency), and
# the last image ends with a small window to shorten the drain tail.
WINDOWS_IMG0 = [(0, 2), (2, 8), (10, 41), (51, 51)]
WINDOWS_STD = [(0, 51), (51, 51)]
WINDOWS_LAST = [(0, 51), (51, 31), (82, 20)]


def _prep_inputs(x, conv_weight, conv_bias):
    """Host-side packing: slab-layout fp8e3 image tensor and fp16 weights.

    x7[i, kw*16+c, t] = x[i, c, t+kw] (kw in 0..6), row 112 = ones — exactly
    the SBUF slab layout, so device loads are single contiguous 113-partition
    DMAs.
    """
    n = x.shape[0]
    xf = x.reshape(n, C_IN, IMG)
    x7 = np.zeros((n, KDIM, PAD_COLS), dtype=ml_dtypes.float8_e3m4)
    for kw in range(WK):
        lo = max(0, IMG - kw)
        x7[:, kw * C_IN:(kw + 1) * C_IN, :lo] = xf[:, :, kw:kw + lo].astype(
            ml_dtypes.float8_e3m4
        )
    x7[:, KDIM - 1, :] = 1.0

    # wmov[kh][kw*16+c, j*64+o] = W[o, c, kh, kw-j] for kw-j in 0..2
    wmov = np.zeros((KDIM, 3, J, C_OUT), dtype=np.float32)
    for kh in range(3):
        for kw in range(WK):
            for j in range(J):
                kk = kw - j
                if 0 <= kk <= 2:
                    wmov[kw * C_IN:(kw + 1) * C_IN, kh, j, :] = (
                        conv_weight[:, :, kh, kk].T
                    )
    wmov[KDIM - 1, 0, :, :] = conv_bias[None, :]  # bias via ones row, kh=0 only
    wmov = wmov.reshape(KDIM, 3 * NFREE).astype(np.float16)
    return x7, wmov


def _build_program():
    nc = bacc.Bacc(
        "TRN2", target_bir_lowering=False, debug=False, num_devices=N_CORES
    )
    f8 = mybir.dt.float8e3
    f16 = mybir.dt.float16
    f32 = mybir.dt.float32

    x_d = nc.dram_tensor(
        "x", [IMGS_PER_CORE, KDIM, PAD_COLS], f8, kind="ExternalInput"
    )
    w_d = nc.dram_tensor("w", [KDIM, 3 * NFREE], f16, kind="ExternalInput")
    # y[i, m, g*J+j] = out at flat pos g*640 + m*5 + j
    y_d = nc.dram_tensor(
        "y", [IMGS_PER_CORE, 128, NBLOCKS * J], f32, kind="ExternalOutput"
    )

    windows = []
    for i in range(IMGS_PER_CORE):
        if i == 0:
            sched = WINDOWS_IMG0
        elif i == IMGS_PER_CORE - 1:
            sched = WINDOWS_LAST
        else:
            sched = WINDOWS_STD
        for sb, nb in sched:
            windows.append((i, sb, nb))

    with tile.TileContext(nc) as tc:
        with (
            tc.tile_pool(name="wpool", bufs=1) as wpool,
            tc.tile_pool(name="slab", bufs=3) as slab_pool,
            tc.tile_pool(name="stage", bufs=2) as stage_pool,
            tc.tile_pool(name="psum", bufs=2, space="PSUM") as psum_pool,
        ):
            w_t = wpool.tile([KDIM, 3 * NFREE], f16)
            nc.sync.dma_start(w_t[:], w_d[:])

            def load_window(idx):
                i, sb, nb = windows[idx]
                wstart = sb * BLK
                wcols = nb * BLK + CEXTRA
                slab = slab_pool.tile([KDIM, wcols], f8)
                # 112-partition contiguous transfers spray all 16 SDMA
                # engines and cost ~0.6us of HWDGE descriptor-gen each.
                nch = 4 if nb > 16 else 1
                chunk = (wcols + nch - 1) // nch
                for h in range(nch):
                    ca = h * chunk
                    cb = min(wcols, ca + chunk)
                    nc.sync.dma_start(
                        slab[0:112, ca:cb],
                        x_d[i, 0:112, wstart + ca:wstart + cb],
                    )
                # The ones row is a [1, N] spray (16 descriptors, ~2.4us of
                # HWDGE descriptor-gen) — issue it AFTER the body chunks so
                # it never delays their descriptor generation.
                nc.sync.dma_start(
                    slab[112:113, :],
                    x_d[i, 112:113, wstart:wstart + wcols],
                )
                return slab

            # 2-deep prefetch (3 slab buffers): window w+2's DMA overlaps
            # computes of w and w+1, hiding the image-0 ramp-up and jitter.
            slabs = [
                load_window(0),
                load_window(1) if len(windows) > 1 else None,
            ]
            for idx in range(len(windows)):
                if idx + 2 < len(windows):
                    slabs.append(load_window(idx + 2))
                else:
                    slabs.append(None)
                slab = slabs[idx]
                i, sb, nb = windows[idx]
                mn = stage_pool.tile([128, nb, J], f32, tag="mn")
                for q0 in range(0, nb, GRP):
                    ng = min(GRP, nb - q0)
                    # bank-aligned PSUM sub-blocks so one DVE reduce covers
                    # GRP blocks, amortizing the ~120-cycle reduce overhead.
                    ps = psum_pool.tile([128, GRP, 512], f32)
                    for s in range(ng):
                        rel = (q0 + s) * BLK  # window-relative col offset
                        for kh in range(3):
                            s0 = rel + kh * W
                            lhsT = (
                                slab[:, s0:s0 + BLK]
                                .rearrange("p (m j) -> p m j", m=128)
                                [:, :, 0:1]
                            )
                            nc.tensor.matmul(
                                ps[:, s, 0:NFREE],
                                lhsT,
                                w_t[:, kh * NFREE:(kh + 1) * NFREE],
                                start=(kh == 0),
                                stop=(kh == 2),
                            )
                    nc.vector.tensor_reduce(
                        mn[:, q0:q0 + ng, :],
                        ps[:, 0:ng, 0:NFREE].rearrange(
                            "p s (j o) -> p s j o", o=C_OUT
                        ),
                        axis=mybir.AxisListType.X,
                        op=mybir.AluOpType.min,
                    )
                th = stage_pool.tile([128, nb * J], f32, tag="th")
                nc.scalar.activation(
                    th[:], mn[:].rearrange("p g j -> p (g j)"),
                    mybir.ActivationFunctionType.Tanh,
                )
                nc.scalar.activation(
                    th[:], th[:],
                    mybir.ActivationFunctionType.Tanh,
                )
                # SWDGE queue: keeps output stores off the Sync FIFO
                # so they never delay the slab prefetch DMAs.
                nc.gpsimd.dma_start(
                    y_d[i, :, sb * J:(sb + nb) * J], th[:]
                )
    nc.compile()
    return nc


_NC_CACHE = []


def _get_nc():
    if not _NC_CACHE:
        _NC_CACHE.append(_build_program())
    return _NC_CACHE[0]


def kernel(x, conv_weight, conv_bias, _trace=False):
    x = np.asarray(x, dtype=np.float32)
    conv_weight = np.asarray(conv_weight, dtype=np.float32)
    conv_bias = np.asarray(conv_bias, dtype=np.float32)
    n = x.shape[0]
    assert n == N_CORES * IMGS_PER_CORE

    x_aug, wmov = _prep_inputs(x, conv_weight, conv_bias)
    nc = _get_nc()
    in_maps = [
        {
            "x": np.ascontiguousarray(
                x_aug[c * IMGS_PER_CORE:(c + 1) * IMGS_PER_CORE]
            ),
            "w": wmov,
        }
        for c in range(N_CORES)
    ]
    res = run_bass_kernel_spmd(
        nc, in_maps, core_ids=list(range(N_CORES)), trace=_trace
    )
    # y core result: [4, 128, 510] -> [4, 102, 128, 5] -> flat [4, 65280]
    y = np.concatenate([r["y"] for r in res.results], axis=0)  # [32,128,510]
    y = y.reshape(n, 128, NBLOCKS, J).transpose(0, 2, 1, 3)
    y = y.reshape(n, NBLOCKS * BLK)
    y = y.reshape(n, 1, 255, 256)[:, :, :OH, :OW]
    out = np.ascontiguousarray(y)
    if _trace:
        kernel._last_result = res
    return out


# revision 17
# speedup vs baseline: 1.0687x; 1.0687x over previous
"""Trainium2 Bass kernel: conv2d(3x3,VALID) + bias -> min over C_out -> tanh(tanh).

Full-input contract: kernel(**inputs) takes the unsharded inputs
  x:           [32, 16, 256, 256] f32
  conv_weight: [64, 16, 3, 3]     f32
  conv_bias:   [64]               f32
and returns [32, 1, 254, 254] f32.

Strategy (data-parallel over batch, 4 images per core on 8 cores):
The conv is cast as matmuls over a "kw-shifted slab" in SBUF:
  slab[kw*16+c, t] = x[c, t+kw]   (kw in 0..6, flattened image cols t)
plus a ones row (partition 112) that carries the bias through the matmul.
For a block of 640 flat positions p = base + 5*m + j (m in 0..127, j in 0..4):
  out[m, (j,o)] = sum_kh  slab[:, base+kh*256+5m].T @ wmov[kh]
with wmov[kh][kw*16+c, j*64+o] = W[o,c,kh,kw-j] (0 when kw-j not in 0..2).
This yields PSUM [128 positions, 5 shifts, 64 channels]; the channel-min is
then a free-dim reduce_min on DVE, followed by tanh(tanh()) on ACT.
Computed on the full 256-wide rows; the host drops the 2 garbage columns/rows.

Numerics: the slab is stored in fp8 E3M4 (x ~ N(0,1) fits the +-15.5 range;
4 mantissa bits measure 1.53e-2 final rel err on the actual jax key(0)
inputs vs the 2e-2 budget).  This halves DMA bytes per partition, which is
the wall: all 16 SDMA engines run at ~13-17 GB/s each with all 8 cores
DMAing concurrently, and even AXI ports (partitions 0..63) carry 8 slab
rows each.  Weights stay fp16 (mixed-dtype matmul; the PE upconverts each
operand independently -- verified exact on HW).

Schedule: the Tile DMA->compute dependency is per-tile, so image 0 ramps up
through small windows (2, 8, 41 blocks) to start the first matmul ~12us in
instead of ~26us (full-window DMA wait).  Keep DMAs single-segment
contiguous: multi-segment APs cost ~2.5-3.2us of HWDGE descriptor
generation each vs ~0.6us contiguous (measured).  PSUM reduces are batched
4 blocks per DVE instruction; outputs staged per window, one DMA each.
"""

import sys
import types

import numpy as np
import ml_dtypes

# ---------------------------------------------------------------------------
# NTFF profile hook registration (the container's antenv stub lacks
# axon_hooks; registering it enables trace=True for profiling runs).
def _install_axon_hooks():
    try:
        import antenv.axon_hooks  # noqa: F401
        return
    except ImportError:
        pass
    try:
        import antenv
        from trn_agent_boot.trn_boot import _ntff_profile_via_ctypes
    except ImportError:
        return
    mod = types.ModuleType("antenv.axon_hooks")
    _hook = [None]
    mod.set_axon_ntff_profile_hook = lambda h: _hook.__setitem__(0, h)
    mod.get_axon_ntff_profile_hook = lambda: _hook[0]
    sys.modules["antenv.axon_hooks"] = mod
    antenv.axon_hooks = mod
    try:
        mod.set_axon_ntff_profile_hook(
            _ntff_profile_via_ctypes("/opt/axon/libaxon_pjrt.so")
        )
    except Exception:
        pass


_install_axon_hooks()

import concourse.bass as bass  # noqa: E402
import concourse.tile as tile  # noqa: E402
from concourse import bacc, mybir  # noqa: E402
from concourse.bass_utils import run_bass_kernel_spmd  # noqa: E402

N_CORES = 8
IMGS_PER_CORE = 4
C_IN, H, W = 16, 256, 256
C_OUT = 64
OH = OW = 254

J = 5                 # position shifts per matmul column group
WK = 7                # kw taps present in the contraction (0..6)
KDIM = WK * C_IN + 1  # 113 partitions: 7 kw-shifts x 16 ch + ones row
NFREE = J * C_OUT     # 320 moving columns
BLK = 128 * J         # 640 flat positions per block
IMG = H * W           # 65536
PAD_COLS = 66560      # 260 rows of 256 (2 extra conv rows + slack)
NBLOCKS = 102         # blocks 0..101 cover flat positions 0..65279

# extra cols a window needs past its blocks: the last block's kh=2 lhsT
# slice spans cols [(nb-1)*640 + 512, nb*640 + 512)
CEXTRA = 512
GRP = 4               # blocks per PSUM tile / DVE reduce

# windows: (start_block, n_blocks) per image; image 0 ramps up small so the
# first matmul isn't gated on a full-window DMA (per-tile dependency).
WINDOWS_IMG0 = [(0, 2), (2, 8), (10, 41), (51, 51)]
WINDOWS_STD = [(0, 51), (51, 51)]


def _prep_inputs(x, conv_weight, conv_bias):
    """Host-side packing: slab-layout fp8e3 image tensor and fp16 weights.

    x7[i, kw*16+c, t] = x[i, c, t+kw] (kw in 0..6), row 112 = ones — exactly
    the SBUF slab layout, so device loads are single contiguous 113-partition
    DMAs.
    """
    n = x.shape[0]
    xf = x.reshape(n, C_IN, IMG)
    x7 = np.zeros((n, KDIM, PAD_COLS), dtype=ml_dtypes.float8_e3m4)
    for kw in range(WK):
        lo = max(0, IMG - kw)
        x7[:, kw * C_IN:(kw + 1) * C_IN, :lo] = xf[:, :, kw:kw + lo].astype(
            ml_dtypes.float8_e3m4
        )
    x7[:, KDIM - 1, :] = 1.0

    # wmov[kh][kw*16+c, j*64+o] = W[o, c, kh, kw-j] for kw-j in 0..2
    wmov = np.zeros((KDIM, 3, J, C_OUT), dtype=np.float32)
    for kh in range(3):
        for kw in range(WK):
            for j in range(J):
                kk = kw - j
                if 0 <= kk <= 2:
                    wmov[kw * C_IN:(kw + 1) * C_IN, kh, j, :] = (
                        conv_weight[:, :, kh, kk].T
                    )
    wmov[KDIM - 1, 0, :, :] = conv_bias[None, :]  # bias via ones row, kh=0 only
    wmov = wmov.reshape(KDIM, 3 * NFREE).astype(np.float16)
    return x7, wmov


def _build_program():
    nc = bacc.Bacc(
        "TRN2", target_bir_lowering=False, debug=False, num_devices=N_CORES
    )
    f8 = mybir.dt.float8e3
    f16 = mybir.dt.float16
    f32 = mybir.dt.float32

    x_d = nc.dram_tensor(
        "x", [IMGS_PER_CORE, KDIM, PAD_COLS], f8, kind="ExternalInput"
    )
    w_d = nc.dram_tensor("w", [KDIM, 3 * NFREE], f16, kind="ExternalInput")
    # y[i, m, g*J+j] = out at flat pos g*640 + m*5 + j
    y_d = nc.dram_tensor(
        "y", [IMGS_PER_CORE, 128, NBLOCKS * J], f32, kind="ExternalOutput"
    )

    windows = []
    for i in range(IMGS_PER_CORE):
        for sb, nb in (WINDOWS_IMG0 if i == 0 else WINDOWS_STD):
            windows.append((i, sb, nb))

    with tile.TileContext(nc) as tc:
        with (
            tc.tile_pool(name="wpool", bufs=1) as wpool,
            tc.tile_pool(name="slab", bufs=2) as slab_pool,
            tc.tile_pool(name="stage", bufs=2) as stage_pool,
            tc.tile_pool(name="psum", bufs=2, space="PSUM") as psum_pool,
        ):
            w_t = wpool.tile([KDIM, 3 * NFREE], f16)
            nc.sync.dma_start(w_t[:], w_d[:])

            def load_window(idx):
                i, sb, nb = windows[idx]
                wstart = sb * BLK
                wcols = nb * BLK + CEXTRA
                slab = slab_pool.tile([KDIM, wcols], f8)
                # The ones row (partition 112, odd-port range) rides free of
                # the even-port wall; issue it first on the FIFO ring.
                nc.sync.dma_start(
                    slab[112:113, :],
                    x_d[i, 112:113, wstart:wstart + wcols],
                )
                # 112-partition contiguous transfers spray all 16 SDMA
                # engines and cost ~0.6us of HWDGE descriptor-gen each.
                nch = 4 if nb > 16 else 1
                chunk = (wcols + nch - 1) // nch
                for h in range(nch):
                    ca = h * chunk
                    cb = min(wcols, ca + chunk)
                    nc.sync.dma_start(
                        slab[0:112, ca:cb],
                        x_d[i, 0:112, wstart + ca:wstart + cb],
                    )
                return slab

            slab = load_window(0)
            for idx in range(len(windows)):
                # Prefetch the next window before touching this one so its
                # DMAs overlap this window's compute (keeps PE warm too).
                slab_next = (
                    load_window(idx + 1) if idx + 1 < len(windows) else None
                )
                i, sb, nb = windows[idx]
                mn = stage_pool.tile([128, nb, J], f32, tag="mn")
                for q0 in range(0, nb, GRP):
                    ng = min(GRP, nb - q0)
                    # bank-aligned PSUM sub-blocks so one DVE reduce covers
                    # GRP blocks, amortizing the ~120-cycle reduce overhead.
                    ps = psum_pool.tile([128, GRP, 512], f32)
                    for s in range(ng):
                        rel = (q0 + s) * BLK  # window-relative col offset
                        for kh in range(3):
                            s0 = rel + kh * W
                            lhsT = (
                                slab[:, s0:s0 + BLK]
                                .rearrange("p (m j) -> p m j", m=128)
                                [:, :, 0:1]
                            )
                            nc.tensor.matmul(
                                ps[:, s, 0:NFREE],
                                lhsT,
                                w_t[:, kh * NFREE:(kh + 1) * NFREE],
                                start=(kh == 0),
                                stop=(kh == 2),
                            )
                    nc.vector.tensor_reduce(
                        mn[:, q0:q0 + ng, :],
                        ps[:, 0:ng, 0:NFREE].rearrange(
                            "p s (j o) -> p s j o", o=C_OUT
                        ),
                        axis=mybir.AxisListType.X,
                        op=mybir.AluOpType.min,
                    )
                th = stage_pool.tile([128, nb * J], f32, tag="th")
                nc.scalar.activation(
                    th[:], mn[:].rearrange("p g j -> p (g j)"),
                    mybir.ActivationFunctionType.Tanh,
                )
                nc.scalar.activation(
                    th[:], th[:],
                    mybir.ActivationFunctionType.Tanh,
                )
                # SWDGE queue: keeps output stores off the Sync FIFO
                # so they never delay the slab prefetch DMAs.
                nc.gpsimd.dma_start(
                    y_d[i, :, sb * J:(sb + nb) * J], th[:]
                )
                slab = slab_next
    nc.compile()
    return nc


_NC_CACHE = []


def _get_nc():
    if not _NC_CACHE:
        _NC_CACHE.append(_build_program())
    return _NC_CACHE[0]


def kernel(x, conv_weight, conv_bias, _trace=False):
    x = np.asarray(x, dtype=np.float32)
    conv_weight = np.asarray(conv_weight, dtype=np.float32)
    conv_bias = np.asarray(conv_bias, dtype=np.float32)
    n = x.shape[0]
    assert n == N_CORES * IMGS_PER_CORE

    x_aug, wmov = _prep_inputs(x, conv_weight, conv_bias)
    nc = _get_nc()
    in_maps = [
        {
            "x": np.ascontiguousarray(
                x_aug[c * IMGS_PER_CORE:(c + 1) * IMGS_PER_CORE]
            ),
            "w": wmov,
        }
        for c in range(N_CORES)
    ]
    res = run_bass_kernel_spmd(
        nc, in_maps, core_ids=list(range(N_CORES)), trace=_trace
    )
    # y core result: [4, 128, 510] -> [4, 102, 128, 5] -> flat [4, 65280]
    y = np.concatenate([r["y"] for r in res.results], axis=0)  # [32,128,510]
    y = y.reshape(n, 128, NBLOCKS, J).transpose(0, 2, 1, 3)
    y = y.reshape(n, NBLOCKS * BLK)
    y = y.reshape(n, 1, 255, 256)[:, :, :OH, :OW]
    out = np.ascontiguousarray(y)
    if _trace:
        kernel._last_result = res
    return out
